# revision 7
# baseline (speedup 1.0000x reference)
# CharRNN Trainium2 kernel.
#
# reference math:
#   x  = emb[input_seq]                      [T, B, EMB]
#   xp = einsum('tbe,he->tbh', x, wax_w)     [T, B, HID]
#   h_t = tanh(h_{t-1} @ waa_w.T + waa_b + xp_t)        (scan over T=2048)
#   preds = einsum('tbh,vh->tbv', hs, wya_w) + wya_b
#
# Strategy (data-parallel over batch, 8 cores x 16 batch each):
#   * Fold embedding+input-projection+bias into one table on the host:
#       embp[v] = emb[v] @ wax_w.T + waa_b        [256, 512]  (bf16)
#     so xp_t[b] = embp[input_seq[t, b]] -- an on-device indirect-DMA row
#     gather, no materialized [T,B,HID] tensor in HBM.
#   * Recurrence layout: h kept TRANSPOSED in SBUF as hT [h(128-part), 4x16]
#     (4 hidden chunks side by side, 16 batch columns each, bf16).
#     Per step, PSUM[h'(128), 4x16] accumulates:
#       - xp via an identity-selector matmul:  lhsT = gathered-embp-rows
#         block [128rows,128h'], rhs = I128 column slice -> injects
#         xp^T[h',b] into PSUM (no transpose ever needed).
#       - W via 16 matmuls: lhsT = waa_w block [k=h-chunk,128 x m=h'-chunk,128]
#         (bf16 => fast-weight-load), rhs = hT chunk [128, 16].
#     Then ScalarE tanh(PSUM) -> bf16 hT for the next step. The hidden state
#     stays [h-on-partitions, batch-on-free] forever, which is exactly the
#     orientation both the recurrence and the output projection consume.
#   * Output projection batched per 32-step body from the SBUF h history:
#     8 matmuls of N=512 (bf16), +wya_b on VectorE, DMA to HBM.
#   * T-loop is a tc.For_i over 64 bodies of 32 unrolled steps; embp gathers
#     for body i+1 are issued inside body i right after their slot frees up,
#     so the gather latency is fully hidden.
import numpy as np
import ml_dtypes

T, B, NCORES = 2048, 128, 8
EMB, HID, NCHARS = 128, 512, 256
BL = B // NCORES          # batch per core = 16
BODY = 32                 # steps per For_i body
NBODY = T // BODY         # 64
GSTEPS = 8                # steps per gather group (8*16 = 128 rows)
GPB = BODY // GSTEPS      # gather groups per body = 4
HC = HID // 128           # hidden chunks = 4
VC = NCHARS // 128        # vocab chunks = 2
HW = HC * BL              # h tile width = 64

_BF = ml_dtypes.bfloat16


def _build_nc():
    import concourse.bass as bass
    import concourse.mybir as mybir
    import concourse.tile as tile
    from concourse import bacc
    from concourse.bass import ds

    f32 = mybir.dt.float32
    bf16 = mybir.dt.bfloat16
    i32 = mybir.dt.int32
    Tanh = mybir.ActivationFunctionType.Tanh

    nc = bacc.Bacc("TRN2", target_bir_lowering=False, debug=False)

    wT_d = nc.dram_tensor("wt", [128, 16 * 128], bf16, kind="ExternalInput")
    wya_d = nc.dram_tensor("wya", [128, 8 * 128], bf16, kind="ExternalInput")
    sel_d = nc.dram_tensor("sel", [128, 128], bf16, kind="ExternalInput")
    embp_d = nc.dram_tensor("embp", [NCHARS, HID], bf16, kind="ExternalInput")
    idx_d = nc.dram_tensor("idx", [128, 4 * NBODY + GPB], i32, kind="ExternalInput")
    wyb_d = nc.dram_tensor("wyb", [128, VC], f32, kind="ExternalInput")
    preds_d = nc.dram_tensor("preds", [T, BL, NCHARS], f32, kind="ExternalOutput")
    hlast_d = nc.dram_tensor("hlast", [BL, HID], f32, kind="ExternalOutput")

    with tile.TileContext(nc) as tc:
        with (
            tc.tile_pool(name="const", bufs=1) as const,
            tc.tile_pool(name="ps", bufs=2, space="PSUM") as psum,
            tc.tile_pool(name="pp", bufs=2, space="PSUM") as pspred,
            tc.tile_pool(name="op", bufs=2) as outp,
        ):
            wT = const.tile([128, 16 * 128], bf16, name="wT")
            wya = const.tile([128, 8 * 128], bf16, name="wyat")
            sel = const.tile([128, 128], bf16, name="selt")
            idx = const.tile([128, 4 * NBODY + GPB], i32, name="idxt")
            wyb = const.tile([128, VC], f32, name="wybt")
            # h history: col group s holds hT *entering* step s; +1 for exit
            hh = const.tile([128, (BODY + 1) * HW], bf16, name="hh")
            idxs = const.tile([128, GPB], i32, name="idxs")
            xpg = [
                const.tile([128, HID], bf16, name=f"xpg{g}") for g in range(GPB)
            ]

            nc.sync.dma_start(wT[:], wT_d[:])
            nc.sync.dma_start(wya[:], wya_d[:])
            nc.sync.dma_start(sel[:], sel_d[:])
            nc.sync.dma_start(idx[:], idx_d[:])
            nc.sync.dma_start(wyb[:], wyb_d[:])
            nc.gpsimd.memset(hh[:, 0:HW], 0.0)
            for g in range(GPB):
                nc.gpsimd.indirect_dma_start(
                    out=xpg[g][:],
                    out_offset=None,
                    in_=embp_d[:],
                    in_offset=bass.IndirectOffsetOnAxis(ap=idx[:, g : g + 1], axis=0),
                )

            with tc.For_i(0, NBODY) as i:
                for s in range(BODY):
                    g = s // GSTEPS
                    j = s % GSTEPS
                    base = s * HW
                    nbase = (s + 1) * HW
                    ps = psum.tile([128, HW], f32, name="ps")
                    for cp in range(HC):
                        po = ps[:, cp * BL : (cp + 1) * BL]
                        # xp^T injection: I128-column-selector picks the 16
                        # gathered rows of this step
                        nc.tensor.matmul(
                            po,
                            xpg[g][:, cp * 128 : (cp + 1) * 128],
                            sel[:, j * BL : (j + 1) * BL],
                            start=True,
                            stop=False,
                        )
                        for c in range(HC):
                            nc.tensor.matmul(
                                po,
                                wT[:, (c * HC + cp) * 128 : (c * HC + cp + 1) * 128],
                                hh[:, base + c * BL : base + (c + 1) * BL],
                                start=False,
                                stop=(c == HC - 1),
                            )
                    for half in range(2):
                        hw2 = HW // 2
                        nc.scalar.activation(
                            hh[:, nbase + half * hw2 : nbase + (half + 1) * hw2],
                            ps[:, half * hw2 : (half + 1) * hw2],
                            Tanh,
                        )
                    if j == GSTEPS - 1:
                        # slot g is done for this body; refill for body i+1.
                        # (index column staged via DVE first: the indirect
                        # DMA's offset AP must be register-free)
                        nc.vector.tensor_copy(
                            idxs[:, g : g + 1], idx[:, ds(i * GPB + (GPB + g), 1)]
                        )
                        nc.gpsimd.indirect_dma_start(
                            out=xpg[g][:],
                            out_offset=None,
                            in_=embp_d[:],
                            in_offset=bass.IndirectOffsetOnAxis(
                                ap=idxs[:, g : g + 1], axis=0
                            ),
                        )

                # output projection for this body's 32 steps
                hv = hh[:, HW : HW + BODY * HW].rearrange(
                    "p (s x) -> p s x", s=BODY
                )
                for vp in range(VC):
                    pp = pspred.tile([128, BODY * BL], f32, name="pp")
                    ppv = pp[:].rearrange("p (s b) -> p s b", s=BODY)
                    for c in range(HC):
                        nc.tensor.matmul(
                            ppv,
                            wya[:, (c * VC + vp) * 128 : (c * VC + vp + 1) * 128],
                            hv[:, :, c * BL : (c + 1) * BL],
                            start=(c == 0),
                            stop=(c == HC - 1),
                        )
                    ot = outp.tile([128, BODY * BL], f32, name="ot")
                    nc.vector.tensor_add(
                        ot[:], pp[:], wyb[:, vp : vp + 1].to_broadcast([128, BODY * BL])
                    )
                    dst = preds_d[ds(i * BODY, BODY), :, vp * 128 : (vp + 1) * 128]
                    nc.sync.dma_start(
                        dst.rearrange("s b p -> p s b"),
                        ot[:].rearrange("p (s b) -> p s b", s=BODY),
                    )
                # carry h to next body
                nc.vector.tensor_copy(hh[:, 0:HW], hh[:, BODY * HW : BODY * HW + HW])

            hl = outp.tile([128, HW], f32, name="hl")
            nc.vector.tensor_copy(hl[:], hh[:, 0:HW])
            for c in range(HC):
                nc.sync.dma_start(
                    hlast_d[:, c * 128 : (c + 1) * 128].rearrange("b p -> p b"),
                    hl[:, c * BL : (c + 1) * BL],
                )
    nc.compile()
    return nc


def _prep_inputs(input_seq, emb, waa_w, waa_b, wax_w, wya_w, wya_b):
    # waa_w blocks: block (c, cp) = waa_w[128cp:128cp+128, 128c:128c+128].T
    Wr = np.ascontiguousarray(waa_w).reshape(HC, 128, HC, 128)  # [cp, m, c, k]
    wT = np.concatenate(
        [Wr[cp, :, c, :].T for c in range(HC) for cp in range(HC)], axis=1
    ).astype(_BF)
    wyr = np.ascontiguousarray(wya_w).reshape(VC, 128, HC, 128)  # [vp, m, c, k]
    wya = np.concatenate(
        [wyr[vp, :, c, :].T for c in range(HC) for vp in range(VC)], axis=1
    ).astype(_BF)
    sel = np.eye(128, dtype=_BF)
    embp = (emb.astype(np.float64) @ wax_w.T.astype(np.float64) + waa_b).astype(
        np.float32
    ).astype(_BF)
    wyb = np.ascontiguousarray(wya_b.astype(np.float32).reshape(VC, 128).T)

    seq = np.asarray(input_seq).astype(np.int32)  # [T, B]
    per_core = []
    for core in range(NCORES):
        flat = np.ascontiguousarray(seq[:, core * BL : (core + 1) * BL]).reshape(-1)
        grp = flat.reshape(T * BL // 128, 128).T  # [128, ngroups]
        grp = np.concatenate(
            [grp, np.zeros((128, GPB), np.int32)], axis=1
        )  # pad for the last body's dead prefetch
        per_core.append(
            {
                "wt": wT,
                "wya": wya,
                "sel": sel,
                "embp": embp,
                "idx": np.ascontiguousarray(grp),
                "wyb": wyb,
            }
        )
    return per_core


_NC_CACHE = {}


def kernel(input_seq, emb, waa_w, waa_b, wax_w, wya_w, wya_b):
    from concourse.bass_utils import run_bass_kernel_spmd

    input_seq = np.asarray(input_seq)
    in_maps = _prep_inputs(
        np.asarray(input_seq),
        np.asarray(emb, np.float32),
        np.asarray(waa_w, np.float32),
        np.asarray(waa_b, np.float32),
        np.asarray(wax_w, np.float32),
        np.asarray(wya_w, np.float32),
        np.asarray(wya_b, np.float32),
    )
    if "nc" not in _NC_CACHE:
        _NC_CACHE["nc"] = _build_nc()
    nc = _NC_CACHE["nc"]

    res = run_bass_kernel_spmd(nc, in_maps, core_ids=list(range(NCORES)))
    preds = np.empty((T, B, NCHARS), np.float32)
    h_last = np.empty((B, HID), np.float32)
    for core in range(NCORES):
        preds[:, core * BL : (core + 1) * BL, :] = res.results[core]["preds"]
        h_last[core * BL : (core + 1) * BL, :] = res.results[core]["hlast"]
    return preds, h_last


# revision 18
# speedup vs baseline: 1.3786x; 1.3786x over previous
# CharRNN Trainium2 kernel.
#
# reference math:
#   x  = emb[input_seq]                      [T, B, EMB]
#   xp = einsum('tbe,he->tbh', x, wax_w)     [T, B, HID]
#   h_t = tanh(h_{t-1} @ waa_w.T + waa_b + xp_t)        (scan over T=2048)
#   preds = einsum('tbh,vh->tbv', hs, wya_w) + wya_b
#
# Strategy (data-parallel over batch, 8 cores x 16 batch each):
#   * Fold embedding+input-projection+bias into one table on the host:
#       embp[v] = emb[v] @ wax_w.T + waa_b        [256, 512]  (bf16)
#     so xp_t[b] = embp[input_seq[t, b]] -- an on-device indirect-DMA row
#     gather, no materialized [T,B,HID] tensor in HBM.
#   * Recurrence layout: h kept TRANSPOSED in SBUF as hT [h(128-part), 4x16]
#     (4 hidden chunks side by side, 16 batch columns each, bf16).
#     Per step, PSUM[h'(128), 4x16] accumulates:
#       - xp via an identity-selector matmul:  lhsT = gathered-embp-rows
#         block [128rows,128h'], rhs = I128 column slice -> injects
#         xp^T[h',b] into PSUM (no transpose ever needed).
#       - W via 16 matmuls: lhsT = waa_w block [k=h-chunk,128 x m=h'-chunk,128]
#         (bf16 => fast-weight-load), rhs = hT chunk [128, 16].
#     Then ScalarE tanh(PSUM) -> bf16 hT for the next step. The hidden state
#     stays [h-on-partitions, batch-on-free] forever, which is exactly the
#     orientation both the recurrence and the output projection consume.
#   * Output projection batched per 32-step body from the SBUF h history:
#     8 matmuls of N=512 (bf16), +wya_b on VectorE, DMA to HBM.
#   * T-loop is a tc.For_i over 64 bodies of 32 unrolled steps; embp gathers
#     for body i+1 are issued inside body i right after their slot frees up,
#     so the gather latency is fully hidden.
import numpy as np
import ml_dtypes

T, B, NCORES = 2048, 128, 8
EMB, HID, NCHARS = 128, 512, 256
BL = B // NCORES          # batch per core = 16
BODY = 32                 # steps per For_i body
NBODY = T // BODY         # 64
GSTEPS = 8                # steps per gather group (8*16 = 128 rows)
GPB = BODY // GSTEPS      # gather groups per body = 4
HC = HID // 128           # hidden chunks = 4
VC = NCHARS // 128        # vocab chunks = 2
HW = HC * BL              # h tile width = 64

_BF = ml_dtypes.bfloat16

REPEAT = 1  # timing only: re-run the whole T loop REPEAT times on device

# ablation flags (timing experiments only; leave all True for correct output)
EN_WMM = True      # the 16 recurrence matmuls
EN_SEL = True      # the 4 xp-injection matmuls
EN_TANH = True     # scalar-engine tanh
EN_GATHER = True   # embp indirect-DMA gathers
EN_PREDS = True    # output projection matmuls
EN_PREDS_DMA = True  # preds DMA to HBM


def _build_nc():
    import concourse.bass as bass
    import concourse.mybir as mybir
    import concourse.tile as tile
    from concourse import bacc
    from concourse.bass import ds

    f32 = mybir.dt.float32
    bf16 = mybir.dt.bfloat16
    i32 = mybir.dt.int32
    Tanh = mybir.ActivationFunctionType.Tanh

    nc = bacc.Bacc("TRN2", target_bir_lowering=False, debug=False)

    wT_d = nc.dram_tensor("wt", [128, 16 * 128], bf16, kind="ExternalInput")
    wya_d = nc.dram_tensor("wya", [128, HC * NCHARS], bf16, kind="ExternalInput")
    sel_d = nc.dram_tensor("sel", [128, GSTEPS * HW], bf16, kind="ExternalInput")
    embp_d = nc.dram_tensor("embp", [NCHARS, HID], bf16, kind="ExternalInput")
    idx_d = nc.dram_tensor("idx", [128, 4 * NBODY + GPB], i32, kind="ExternalInput")
    wyb_d = nc.dram_tensor("wyb", [128, NCHARS], f32, kind="ExternalInput")
    preds_d = nc.dram_tensor("preds", [T, BL, NCHARS], f32, kind="ExternalOutput")
    hlast_d = nc.dram_tensor("hlast", [BL, HID], f32, kind="ExternalOutput")

    with tile.TileContext(nc) as tc:
        with (
            tc.tile_pool(name="const", bufs=1) as const,
            tc.tile_pool(name="ps", bufs=2, space="PSUM") as psum,
            tc.tile_pool(name="pp", bufs=2, space="PSUM") as pspred,
            tc.tile_pool(name="op", bufs=2) as outp,
        ):
            wT = const.tile([128, 16 * 128], bf16, name="wT")
            wya = const.tile([128, HC * NCHARS], bf16, name="wyat")
            sel = const.tile([128, GSTEPS * HW], bf16, name="selt")
            idx = const.tile([128, 4 * NBODY + GPB], i32, name="idxt")
            wyb = const.tile([128, NCHARS], f32, name="wybt")
            # h history: col group s holds hT *entering* step s; +1 for exit
            hh = const.tile([128, (BODY + 1) * HW], bf16, name="hh")
            idxs = const.tile([128, GPB], i32, name="idxs")
            xpg = [
                const.tile([128, HID], bf16, name=f"xpg{g}") for g in range(GPB)
            ]

            nc.sync.dma_start(wT[:], wT_d[:])
            nc.sync.dma_start(wya[:], wya_d[:])
            nc.sync.dma_start(sel[:], sel_d[:])
            nc.sync.dma_start(idx[:], idx_d[:])
            nc.sync.dma_start(wyb[:], wyb_d[:])
            nc.gpsimd.memset(hh[:, 0:HW], 0.0)
            for g in range(GPB):
                nc.gpsimd.indirect_dma_start(
                    out=xpg[g][:],
                    out_offset=None,
                    in_=embp_d[:],
                    in_offset=bass.IndirectOffsetOnAxis(ap=idx[:, g : g + 1], axis=0),
                )

            rep_ctx = tc.For_i(0, REPEAT) if REPEAT > 1 else None
            if rep_ctx is not None:
                rep_ctx.__enter__()
            with tc.For_i(0, NBODY) as i:
                for s in range(BODY):
                    g = s // GSTEPS
                    j = s % GSTEPS
                    base = s * HW
                    nbase = (s + 1) * HW
                    # PSUM columns ordered exactly like hh -- (b4, c', r) --
                    # so tanh is one contiguous [128, 64] ACT op.
                    ps = psum.tile([128, HW], f32, name="ps")
                    ps4 = ps[:].rearrange("p (b4 cc r) -> p cc b4 r", b4=4, cc=HC, r=4)
                    hin = hh[:, base : base + HW].rearrange(
                        "p (b4 cc r) -> p cc b4 r", b4=4, cc=HC, r=4
                    )
                    for cp in range(HC):
                        po = ps4[:, cp]
                        # xp^T injection: I128-column-selector picks the 16
                        # gathered rows of this step
                        if EN_SEL:
                            nc.tensor.matmul(
                                po,
                                xpg[g][:, cp * 128 : (cp + 1) * 128],
                                sel[:, j * HW : (j + 1) * HW].rearrange(
                                    "p (b4 cc r) -> p cc b4 r", b4=4, cc=HC, r=4
                                )[:, 0],
                                start=True,
                                stop=not EN_WMM,
                            )
                        if EN_WMM:
                            for c in range(HC):
                                nc.tensor.matmul(
                                    po,
                                    wT[:, (c * HC + cp) * 128 : (c * HC + cp + 1) * 128],
                                    hin[:, c],
                                    start=(not EN_SEL) and c == 0,
                                    stop=(c == HC - 1),
                                )
                    if EN_TANH:
                        nc.scalar.activation(
                            hh[:, nbase : nbase + HW], ps[:], Tanh
                        )
                    else:
                        nc.vector.tensor_copy(hh[:, nbase : nbase + HW], ps[:])
                    if j == GSTEPS - 1 and EN_GATHER:
                        # slot g is done for this body; refill for body i+1.
                        # (index column staged via DVE first: the indirect
                        # DMA's offset AP must be register-free)
                        nc.vector.tensor_copy(
                            idxs[:, g : g + 1], idx[:, ds(i * GPB + (GPB + g), 1)]
                        )
                        nc.gpsimd.indirect_dma_start(
                            out=xpg[g][:],
                            out_offset=None,
                            in_=embp_d[:],
                            in_offset=bass.IndirectOffsetOnAxis(
                                ap=idxs[:, g : g + 1], axis=0
                            ),
                        )

                # Output projection, (step,batch)-on-partitions so the HBM
                # write is contiguous: psum_r partition m holds pair
                # q = 4m + r (q = 16s + b), i.e. partition p of the staging
                # tile owns a contiguous 4KB run of preds[t0:t0+32].
                if EN_PREDS:
                    hv4 = hh[:, HW : HW + BODY * HW].rearrange(
                        "p (q4 cc r) -> p cc r q4", q4=BODY * 4, cc=HC, r=4
                    )
                    ot = outp.tile([128, 4 * NCHARS], f32, name="ot")
                    for r in range(4):
                        pq = pspred.tile([128, NCHARS], f32, name="pq")
                        for c in range(HC):
                            nc.tensor.matmul(
                                pq[:],
                                hv4[:, c, r],
                                wya[:, c * NCHARS : (c + 1) * NCHARS],
                                start=(c == 0),
                                stop=(c == HC - 1),
                            )
                        nc.vector.tensor_add(
                            ot[:, r * NCHARS : (r + 1) * NCHARS], pq[:], wyb[:]
                        )
                    if EN_PREDS_DMA:
                        dst = preds_d[ds(i * BODY, BODY), :, :]
                        nc.sync.dma_start(
                            dst.rearrange("s (bh bs) v -> (s bh) bs v", bs=4),
                            ot[:].rearrange("p (bs v) -> p bs v", bs=4),
                        )
                # carry h to next body
                nc.vector.tensor_copy(hh[:, 0:HW], hh[:, BODY * HW : BODY * HW + HW])
            if rep_ctx is not None:
                rep_ctx.__exit__(None, None, None)

            # de-permute into natural (c, b) order during the fp32 upcast copy
            hl = outp.tile([128, HW], f32, name="hl")
            hhv = hh[:, 0:HW].rearrange("p (b4 cc r) -> p cc b4 r", b4=4, cc=HC, r=4)
            hlv = hl[:].rearrange("p (cc b4 r) -> p cc b4 r", cc=HC, b4=4, r=4)
            for c in range(HC):
                nc.vector.tensor_copy(hlv[:, c], hhv[:, c])
            for c in range(HC):
                nc.sync.dma_start(
                    hlast_d[:, c * 128 : (c + 1) * 128].rearrange("b p -> p b"),
                    hl[:, c * BL : (c + 1) * BL],
                )
    nc.compile()
    return nc


def _prep_inputs(input_seq, emb, waa_w, waa_b, wax_w, wya_w, wya_b):
    # waa_w blocks: block (c, cp) = waa_w[128cp:128cp+128, 128c:128c+128].T
    Wr = np.ascontiguousarray(waa_w).reshape(HC, 128, HC, 128)  # [cp, m, c, k]
    wT = np.concatenate(
        [Wr[cp, :, c, :].T for c in range(HC) for cp in range(HC)], axis=1
    ).astype(_BF)
    # wya as moving operand: block c holds wya_w[:, 128c:128c+128].T
    wya = np.concatenate(
        [np.ascontiguousarray(wya_w[:, c * 128 : (c + 1) * 128]).T for c in range(HC)],
        axis=1,
    ).astype(_BF)
    sel = np.zeros((128, GSTEPS * HW), np.float32)
    for jj in range(GSTEPS):
        for b in range(BL):
            sel[16 * jj + b, 64 * jj + 16 * (b // 4) + (b % 4)] = 1.0
    sel = sel.astype(_BF)
    embp = (emb.astype(np.float64) @ wax_w.T.astype(np.float64) + waa_b).astype(
        np.float32
    ).astype(_BF)
    wyb = np.ascontiguousarray(
        np.broadcast_to(wya_b.astype(np.float32)[None, :], (128, NCHARS))
    )

    seq = np.asarray(input_seq).astype(np.int32)  # [T, B]
    per_core = []
    for core in range(NCORES):
        flat = np.ascontiguousarray(seq[:, core * BL : (core + 1) * BL]).reshape(-1)
        grp = flat.reshape(T * BL // 128, 128).T  # [128, ngroups]
        grp = np.concatenate(
            [grp, np.zeros((128, GPB), np.int32)], axis=1
        )  # pad for the last body's dead prefetch
        per_core.append(
            {
                "wt": wT,
                "wya": wya,
                "sel": sel,
                "embp": embp,
                "idx": np.ascontiguousarray(grp),
                "wyb": wyb,
            }
        )
    return per_core


_NC_CACHE = {}


def kernel(input_seq, emb, waa_w, waa_b, wax_w, wya_w, wya_b):
    from concourse.bass_utils import run_bass_kernel_spmd

    input_seq = np.asarray(input_seq)
    in_maps = _prep_inputs(
        np.asarray(input_seq),
        np.asarray(emb, np.float32),
        np.asarray(waa_w, np.float32),
        np.asarray(waa_b, np.float32),
        np.asarray(wax_w, np.float32),
        np.asarray(wya_w, np.float32),
        np.asarray(wya_b, np.float32),
    )
    if "nc" not in _NC_CACHE:
        _NC_CACHE["nc"] = _build_nc()
    nc = _NC_CACHE["nc"]

    res = run_bass_kernel_spmd(nc, in_maps, core_ids=list(range(NCORES)))
    preds = np.empty((T, B, NCHARS), np.float32)
    h_last = np.empty((B, HID), np.float32)
    for core in range(NCORES):
        preds[:, core * BL : (core + 1) * BL, :] = res.results[core]["preds"]
        h_last[core * BL : (core + 1) * BL, :] = res.results[core]["hlast"]
    return preds, h_last
